# revision 30
# baseline (speedup 1.0000x reference)
"""Distributed causal multi-head attention for 8 TRN2 NeuronCores.

Problem: B=4, S=2048, D=1024, H=16 heads of DH=64, fp32, causal + padding mask.

Sharding: core c -> (batch b = c//2, head-group g = c%2 of 8 heads).

v3 design (v1 phase-serial: 530us, v2 interleaved: 439us):
  * All inputs arrive fp16 (halves DMA + SBUF, 1 cycle/row matmuls at any span).
  * attT stays in SBUF; output projection reads it directly (no DRAM bounce).
  * Q/K/V projections are emitted as "filler" matmul groups interleaved into
    the per-head attention i-loops, so the PE never idles while the scalar
    engine runs the softmax exp stream (PE idle windows let the HAM clock
    gate drop the PE to 1.2 GHz).
  * Each head runs in two q-half passes (q<1024, q>=1024) so only two AV
    chunk accumulators are live at once.  PSUM: proj acc P (2 banks),
    scores S (2 banks x2 bufs), AV accs (1 bank x2) = 8 banks.
  * Softmax denominators take the baseline's DMA-reshape to (128,4) before
    the DVE reciprocal: a (1,512) reciprocal runs on ONE lane (3.3us!) and
    clogs the DVE FIFO, stalling the copies that release AV PSUM banks.
  * Output projection for chunks c<2 (q<1024, complete once head 7 pass A
    is normalized) interleaves into head 7 pass B; only c>=2 is tail work.

Per-core math for its (batch, group) with X as (dims x seq) fp16:
    qt[j] = Wq_pair_j @ XqT    (128, 2048)  heads 2j / 2j+1 on partitions
    kt[j] = Wk_pair_j @ XkvT   (128, 2048)
    vt[i] = Xkv_tile_i @ Wv^T  (128 keys, 8*(64+1)) with ones column per head
    per head h, per q-half: S^T tiles (keys x q), exp with pad bias, causal
    via affine_select on the diagonal tile, AV into (65 x 512) chunk accs;
    row 64 = softmax denominators (ones-column trick); normalize on DVE.
    outT_partial = Wo_gT @ attT  (1024, 2048) fp16; host sums the two
    per-batch partials in fp32 and transposes.
"""

from collections import deque

import numpy as np

import concourse.bass as bass
import concourse.mybir as mybir
import concourse.tile as tile
from concourse import bacc

B, S, D, H = 4, 2048, 1024, 16
DH = 64
NG = 2              # head groups (cores per batch)
DG = D // NG        # 512 head dims per core
HL = H // NG        # 8 heads per core
PB = 128            # partition block
CH = 512            # fp32 PSUM bank in elements
NKT = S // PB       # 16 key tiles
NDT = D // PB       # 8 contraction tiles for projections
NJT = DG // PB      # 4 head-pair tiles per core
HS = S // 2         # 1024 = one q-half
F32 = mybir.dt.float32
F16 = mybir.dt.float16
SCALE = 1.0 / 8.0   # 1/sqrt(DH)
EXP = mybir.ActivationFunctionType.Exp


def _emit(nc, xq, xkv, wq, wk, wv, wo, pb, outT):
    with tile.TileContext(nc) as tc:
        with (
            tc.tile_pool(name="pers", bufs=1) as pers,
            tc.tile_pool(name="xp", bufs=1) as xp,
            tc.tile_pool(name="wp", bufs=1) as wp,
            tc.tile_pool(name="act", bufs=1) as actp,
            tc.tile_pool(name="ex", bufs=4) as exp_pool,
            tc.tile_pool(name="nrm", bufs=3) as nrmp,
            tc.tile_pool(name="ost", bufs=3) as ostp,
            tc.tile_pool(name="ps", bufs=1, space="PSUM") as ps,
        ):
            # ---------------- persistent small tiles ----------------
            pbias_sb = pers.tile([PB, NKT], F32, tag="pbias", name="pbias_sb")
            nc.sync.dma_start(out=pbias_sb[:], in_=pb[:].rearrange("(i p) -> p i", p=PB))

            # pre-warm the ACT exp table during the DMA prologue
            dummy = pers.tile([1, 8], F32, tag="dummy", name="dummy")
            nc.gpsimd.memset(dummy[:], 0.0)
            nc.scalar.activation(dummy[:], dummy[:], EXP)

            # ---------------- input tiles (persistent, fp16) ----------------
            # one wide tile per tensor, d-tiles as views, so the whole tensor
            # loads in 1-2 folded DMAs (each dma_start costs ~0.6us of
            # Sync-engine issue time; 40+ per-tile loads serialized the
            # prologue on issue rate alone)
            xq_all = xp.tile([PB, NDT * S], F16, tag="xqa", name="xq_all")
            xk_all = xp.tile([PB, NDT * S], F16, tag="xka", name="xk_all")
            wq_all = wp.tile([PB, NDT * DG], F16, tag="wqa", name="wq_all")
            wk_all = wp.tile([PB, NDT * DG], F16, tag="wka", name="wk_all")
            wv_all = wp.tile([PB, NDT * DG], F16, tag="wva", name="wv_all")
            wo_all = wp.tile([PB, NJT * D], F16, tag="woa", name="wo_all")
            xqt = [xq_all[:, d * S:(d + 1) * S] for d in range(NDT)]
            xkt = [xk_all[:, d * S:(d + 1) * S] for d in range(NDT)]
            wqt = [wq_all[:, d * DG:(d + 1) * DG] for d in range(NDT)]
            wkt = [wk_all[:, d * DG:(d + 1) * DG] for d in range(NDT)]
            wvt = [wv_all[:, d * DG:(d + 1) * DG] for d in range(NDT)]
            wot = [wo_all[:, j * D:(j + 1) * D] for j in range(NJT)]

            def load_w(dst_all, src, width):
                nc.sync.dma_start(
                    out=dst_all[:].rearrange("p (d w) -> p d w", w=width),
                    in_=src[:].rearrange("(d p) w -> p d w", p=PB))

            def load_x(dst_all, src, half, dlo, dhi):
                nc.sync.dma_start(
                    out=dst_all[:].rearrange("p (d s) -> p d s", s=S)[
                        :, dlo:dhi, half * HS:(half + 1) * HS],
                    in_=src[dlo * PB:dhi * PB, half * HS:(half + 1) * HS].rearrange(
                        "(d p) s -> p d s", p=PB))

            load_w(wq_all, wq, DG)
            load_x(xq_all, xq, 0, 0, 4)
            load_x(xq_all, xq, 0, 4, 8)
            load_w(wk_all, wk, DG)
            load_x(xk_all, xkv, 0, 0, 4)
            load_x(xk_all, xkv, 0, 4, 8)
            load_w(wv_all, wv, DG)
            load_x(xq_all, xq, 1, 0, 8)
            load_x(xk_all, xkv, 1, 0, 8)
            load_w(wo_all, wo, D)

            # ---------------- long-lived activation tiles ----------------
            qt = [actp.tile([PB, S], F16, tag=f"qt{j}", name=f"qt{j}") for j in range(NJT)]
            kt = [actp.tile([PB, S], F16, tag=f"kt{j}", name=f"kt{j}") for j in range(NJT)]
            vt = [actp.tile([PB, HL * (DH + 1)], F16, tag=f"vt{i}", name=f"vt{i}") for i in range(NKT)]
            att = [actp.tile([PB, S], F16, tag=f"at{j}", name=f"att{j}") for j in range(NJT)]
            ones8 = pers.tile([PB, HL], F32, tag="ones8", name="ones8")
            nc.gpsimd.memset(ones8[:], 1.0)
            for i in range(NKT):
                ones_view = vt[i][:].rearrange("p (h c) -> p h c", c=DH + 1)[:, :, DH]
                nc.vector.tensor_copy(ones_view, ones8[:])

            # ---------------- PSUM tiles ----------------
            def p_acc():  # (128,1024) f32, 2 banks, single buffer
                return ps.tile([PB, 2 * CH], F32, tag="P", name="p_acc")

            def s_tile():  # scores, (128,1024) f32, 2 banks, double buffered
                return ps.tile([PB, HS], F32, tag="S", bufs=2, name="s_tile")

            def av_tile(k):  # AV chunk acc, 1 bank
                return ps.tile([PB, CH], F32, tag=f"AV{k}", name=f"av{k}")

            # ---------------- projection rounds ----------------
            def qk_round(w_tiles, x_tiles, dst, j, rh, acc):
                # one q-half of one head-pair projection: 16 matmuls + 1 copy
                for d in range(NDT):
                    for cc in range(2):
                        nc.tensor.matmul(
                            acc[:, cc * CH:(cc + 1) * CH],
                            w_tiles[d][:, j * PB:(j + 1) * PB],
                            x_tiles[d][:, rh * HS + cc * CH:rh * HS + (cc + 1) * CH],
                            start=(d == 0), stop=(d == NDT - 1),
                        )
                nc.vector.tensor_copy(dst[:, rh * HS:(rh + 1) * HS], acc[:])

            def v_pass(p, acc):
                # key tiles 2p, 2p+1 for all 8 heads: 16 matmuls + 2 copies
                for d in range(NDT):
                    for kk in range(2):
                        i = 2 * p + kk
                        nc.tensor.matmul(
                            acc[:, kk * CH:(kk + 1) * CH],
                            xkt[d][:, i * PB:(i + 1) * PB],
                            wvt[d][:],
                            start=(d == 0), stop=(d == NDT - 1),
                        )
                for kk in range(2):
                    i = 2 * p + kk
                    src = acc[:, kk * CH:(kk + 1) * CH].rearrange("p (h c) -> p h c", c=DH)
                    dst = vt[i][:].rearrange("p (h c) -> p h c", c=DH + 1)[:, :, 0:DH]
                    nc.vector.tensor_copy(dst, src)

            def out_proj(m, c, acc):
                for j in range(NJT):
                    nc.tensor.matmul(
                        acc,
                        wot[j][:, m * PB:(m + 1) * PB],
                        att[j][:, c * CH:(c + 1) * CH],
                        start=(j == 0), stop=(j == NJT - 1),
                    )
                ost = ostp.tile([PB, CH], F16, tag="ost", name="ost")
                nc.vector.tensor_copy(ost[:], acc)
                nc.sync.dma_start(
                    out=outT[m * PB:(m + 1) * PB, c * CH:(c + 1) * CH],
                    in_=ost[:])

            def out_proj_pair(m0, c0, m1, c1, acc=None):
                acc = p_acc() if acc is None else acc
                out_proj(m0, c0, acc[:, 0:CH])
                out_proj(m1, c1, acc[:, CH:2 * CH])

            # HAM warm-up: the PE clock sits at 1.2 GHz until ~3.4us of
            # sustained activity.  Burn dummy matmuls on a memset tile while
            # the input DMAs stream so the real prologue runs at 2.4 GHz.
            warm = pers.tile([PB, CH], F16, tag="warm", name="warm")
            nc.gpsimd.memset(warm[:], 0.0)
            wacc = p_acc()
            for k in range(48):
                nc.tensor.matmul(
                    wacc[:, (k % 2) * CH:(k % 2 + 1) * CH],
                    warm[:, 0:PB], warm[:],
                    start=True, stop=True,
                )

            # prologue: minimum to start head 0 pass A (qt[0]/kt[0] q-half0;
            # vt[0..1] arrives via the i=0 pop); all else is interleaved filler.
            qk_round(wqt, xqt, qt[0], 0, 0, p_acc())
            qk_round(wkt, xkt, kt[0], 0, 0, s_tile())

            # all heads run pass A (q<1024) first, then all run pass B: the
            # q<1024 output-projection columns unlock at the START of phase B
            # and become its PE filler, and phase A hosts the projections.
            filler = deque()
            for p in range(4):
                filler.append(lambda p=p: v_pass(p, p_acc()))
            for j in range(1, NJT):
                filler.append(lambda j=j: qk_round(wqt, xqt, qt[j], j, 0, p_acc()))
                filler.append(lambda j=j: qk_round(wkt, xkt, kt[j], j, 0, p_acc()))
            filler.append(lambda: qk_round(wqt, xqt, qt[0], 0, 1, p_acc()))
            filler.append(lambda: qk_round(wkt, xkt, kt[0], 0, 1, p_acc()))
            for p in range(4, 8):
                filler.append(lambda p=p: v_pass(p, p_acc()))
            for j in range(1, NJT):
                filler.append(lambda j=j: qk_round(wqt, xqt, qt[j], j, 1, p_acc()))
                filler.append(lambda j=j: qk_round(wkt, xkt, kt[j], j, 1, p_acc()))

            def pop_filler():
                if filler:
                    filler.popleft()()

            # pop points, hand-placed against data deadlines (a pop's matmuls
            # must be EMITTED before the first instruction that reads them —
            # the PE executes its queue in emission order).  Phase A (half=0)
            # draws V0-V3, the pair r0 rounds, pair-0 r1 and V4-V7; phase B
            # draws the remaining r1 rounds and the c<2 output projection.
            # phase A is PE-bound while phase B is ACT-serial-bound with ~0.5us
            # of spare PE capacity per iteration, so everything without a
            # phase-A deadline (V4-V7, the pair r1 rounds, the unlocked output
            # projection) pops inside phase B
            POPS = {
                (0, 0): (0, 2, 4, 6),   # V0..V3
                (1, 0): (2, 5),         # Qp1r0, Kp1r0
                (2, 0): (2, 5),         # Qp2r0, Kp2r0
                (3, 0): (2, 5),         # Qp3r0, Kp3r0
                (4, 0): (2, 5),         # Qp0r1, Kp0r1
                (5, 0): (4,),           # V4  (phase A passes 5-7 have spare
                (6, 0): (4,),           # V5   PE capacity; B0 was overloaded)
                (7, 0): (2, 5),         # V6, V7
                (0, 1): (5, 11),        # Qp1r1, Kp1r1
                (1, 1): (5, 11),        # Qp2r1, Kp2r1
                (2, 1): (5, 11),        # Qp3r1, Kp3r1
                (3, 1): (5, 13),        # out c<2
                (4, 1): (5, 13),        # out c<2
                (5, 1): (5, 13),        # out c<2
                (6, 1): (5, 13),        # out c<2
                (7, 1): (12, 13, 14, 15),  # out c=2
            }

            # ---------------- attention ----------------
            def attn_pass(h, half, carry):
                j, rowo = h // 2, (h % 2) * DH
                q0 = half * HS
                cbase = half * 2
                avs = [av_tile(0), av_tile(1)]
                stg_h = nrmp.tile([DH, HS], F16, tag="stgh", name="stg_h")
                pops = POPS.get((h, half), ())
                ihi = 8 if half == 0 else 16
                # the last pass's normalization chains gate the closing output
                # projections; issue their small DMAs from the Scalar engine's
                # hardware queue (idle once the exp stream ends) instead of
                # the Sync queue, which is congested with outT stores then
                dma = nc.scalar if (h, half) == (HL - 1, 1) else nc.sync

                def emit_av(i, ex_t):
                    s0 = max(q0, i * PB) - q0
                    for cc in range(1, -1, -1):
                        c = cbase + cc
                        if i > 4 * c + 3:
                            continue
                        if i // 4 == c:
                            off = i * PB - c * CH
                            out_ap = avs[cc][0:DH + 1, off:CH]
                            rhs = ex_t[:, s0:(cc + 1) * CH]
                        else:
                            out_ap = avs[cc][0:DH + 1, :]
                            rhs = ex_t[:, cc * CH:(cc + 1) * CH]
                        nc.tensor.matmul(
                            out_ap,
                            vt[i][:, h * (DH + 1):(h + 1) * (DH + 1)],
                            rhs,
                            start=(i == 0), stop=(i == 4 * c + 3),
                        )
                        if i == 4 * c + 3:
                            # normalize: copy out (frees the psum bank); spread
                            # the denominator row over 128 partitions via DMA so
                            # the reciprocal uses all DVE lanes, broadcast, mult
                            stg = nrmp.tile([DH + 1, CH], F32, tag="stg", name="stg")
                            nc.vector.tensor_copy(stg[:], avs[cc][0:DH + 1, :])
                            dnp = nrmp.tile([PB, 4], F32, tag="dnp", name="dnp")
                            dma.dma_start(out=dnp[:], in_=stg[DH:DH + 1, :])
                            rcs = nrmp.tile([PB, 4], F32, tag="rcs", name="rcs")
                            nc.vector.reciprocal(rcs[:], dnp[:])
                            rc2 = nrmp.tile([1, CH], F32, tag="rc2", name="rc2")
                            dma.dma_start(out=rc2[:], in_=rcs[:])
                            bc = nrmp.tile([DH, CH], F32, tag="bc", name="bc")
                            nc.gpsimd.partition_broadcast(bc[:], rc2[0:1, :])
                            nc.vector.tensor_tensor(
                                stg_h[:, cc * CH:(cc + 1) * CH],
                                stg[0:DH, :], bc[:],
                                mybir.AluOpType.mult,
                            )
                            # ship this chunk to attT now so downstream output
                            # projection unlocks as early as possible
                            dma.dma_start(
                                out=att[j][rowo:rowo + DH,
                                           q0 + cc * CH:q0 + (cc + 1) * CH],
                                in_=stg_h[:, cc * CH:(cc + 1) * CH])

                new_carry = []
                for i in range(ihi):
                    s0 = max(q0, i * PB) - q0      # local causal start in [0,1024)
                    st = s_tile()
                    for cc in range(s0 // CH, 2):
                        lo = max(s0, cc * CH)
                        nc.tensor.matmul(
                            st[:, lo:(cc + 1) * CH],
                            kt[j][rowo:rowo + DH, i * PB:(i + 1) * PB],
                            qt[j][rowo:rowo + DH, q0 + lo:q0 + (cc + 1) * CH],
                            start=True, stop=True,
                        )
                    if i in pops:
                        pop_filler()
                    ex_t = exp_pool.tile([PB, HS], F16, tag="ex", name="ex_t")
                    nc.scalar.activation(
                        ex_t[:, s0:HS], st[:, s0:HS], EXP,
                        bias=pbias_sb[:, i:i + 1], scale=SCALE,
                    )
                    if q0 <= i * PB:
                        # zero q < k inside the 128-wide diagonal block
                        nc.gpsimd.affine_select(
                            out=ex_t[:, s0:s0 + PB], in_=ex_t[:, s0:s0 + PB],
                            compare_op=mybir.AluOpType.is_ge, fill=0.0,
                            base=0, pattern=[[1, PB]],
                            channel_multiplier=-1,
                        )
                    if i == 0 and carry:
                        # previous pass's last AV groups, deferred to here: the
                        # PE covers their exp-latency with this pass's first
                        # scores, and the ACT queue never drains at boundaries
                        for f in carry:
                            f()
                        carry = None
                    if i >= ihi - 2:
                        new_carry.append(lambda i=i, ex=ex_t: emit_av(i, ex))
                    else:
                        emit_av(i, ex_t)
                return new_carry

            carry = []
            for h in range(HL):
                carry = attn_pass(h, 0, carry)
            # q<1024 attT columns complete once phase A drains: output proj
            # c in {0,1} becomes the spread-out filler for phase B, and c=2
            # (complete at h7 pass B i=11) fills its closing iterations
            for m in range(0, NDT, 2):
                for c in range(2):
                    filler.append(lambda m=m, c=c: out_proj_pair(m, c, m + 1, c))
            for m in range(0, NDT, 2):
                filler.append(lambda m=m: out_proj_pair(m, 2, m + 1, 2))
            for h in range(HL):
                carry = attn_pass(h, 1, carry)
            for f in carry:
                f()
            while filler:
                pop_filler()

            # ---------------- output projection, q >= 1536 ----------------
            # rotate over 6 psum accs (P/S halves + both AV banks) so the
            # WAR on the psum->sbuf copies never stalls the matmul stream
            accs6 = []
            for k, m in enumerate(range(0, NDT, 2)):
                if k % 3 == 0:
                    accs6 = [p_acc(), s_tile(), av_tile(0), av_tile(1)]
                if k % 3 < 2:
                    big = accs6[k % 3]
                    out_proj(m, 3, big[:, 0:CH])
                    out_proj(m + 1, 3, big[:, CH:2 * CH])
                else:
                    out_proj(m, 3, accs6[2][:])
                    out_proj(m + 1, 3, accs6[3][:])


def build_module():
    nc = bacc.Bacc()
    xq = nc.declare_dram_parameter("xqT", [D, S], F16, isOutput=False)
    xkv = nc.declare_dram_parameter("xkvT", [D, S], F16, isOutput=False)
    wq = nc.declare_dram_parameter("wqT", [D, DG], F16, isOutput=False)
    wk = nc.declare_dram_parameter("wkT", [D, DG], F16, isOutput=False)
    wv = nc.declare_dram_parameter("wvT", [D, DG], F16, isOutput=False)
    wo = nc.declare_dram_parameter("woT", [DG, D], F16, isOutput=False)
    pb = nc.declare_dram_parameter("pbias", [S], F32, isOutput=False)
    outT = nc.declare_dram_parameter("outT", [D, S], F16, isOutput=True)
    _emit(nc, xq, xkv, wq, wk, wv, wo, pb, outT)
    nc.finalize()
    return nc


_NC = None


def _get_nc():
    global _NC
    if _NC is None:
        _NC = build_module()
    return _NC


def make_in_maps(q_raw, kv_raw, padding_mask, Wq, Wk, Wv, Wo):
    q_raw = np.asarray(q_raw, np.float32)
    kv_raw = np.asarray(kv_raw, np.float32)
    qT = np.ascontiguousarray(q_raw.transpose(0, 2, 1)).astype(np.float16)
    kvT = np.ascontiguousarray(kv_raw.transpose(0, 2, 1)).astype(np.float16)
    pbias = np.where(np.asarray(padding_mask) == 0, -1e9, 0.0).astype(np.float32)
    Wq, Wk, Wv, Wo = (np.asarray(w, np.float32) for w in (Wq, Wk, Wv, Wo))
    wqT = [np.ascontiguousarray(Wq[g * DG:(g + 1) * DG, :].T).astype(np.float16) for g in range(NG)]
    wkT = [np.ascontiguousarray(Wk[g * DG:(g + 1) * DG, :].T).astype(np.float16) for g in range(NG)]
    wvT = [np.ascontiguousarray(Wv[g * DG:(g + 1) * DG, :].T).astype(np.float16) for g in range(NG)]
    woT = [np.ascontiguousarray(Wo[:, g * DG:(g + 1) * DG].T).astype(np.float16) for g in range(NG)]
    in_maps = []
    for c in range(NG * B):
        b, g = divmod(c, NG)
        in_maps.append({
            "xqT": qT[b], "xkvT": kvT[b],
            "wqT": wqT[g], "wkT": wkT[g], "wvT": wvT[g], "woT": woT[g],
            "pbias": pbias[b],
        })
    return in_maps


def gather(results):
    out = np.empty((B, S, D), np.float32)
    for b in range(B):
        out[b] = (results[NG * b]["outT"].astype(np.float32)
                  + results[NG * b + 1]["outT"].astype(np.float32)).T
    return out


def kernel(q_raw, kv_raw, padding_mask, Wq, Wk, Wv, Wo):
    from concourse.bass_utils import run_bass_kernel_spmd

    nc = _get_nc()
    in_maps = make_in_maps(q_raw, kv_raw, padding_mask, Wq, Wk, Wv, Wo)
    res = run_bass_kernel_spmd(nc, in_maps, core_ids=list(range(NG * B)))
    return gather(res.results)
